# revision 31
# baseline (speedup 1.0000x reference)
"""Trainium2 Bass kernel for multi-head global attention (the
"DeformableAttention" module whose relative-position-bias path is inactive).

Reference computation (per batch b):
    qkv = x @ w_qkv.T + b_qkv            # [N, 3C]
    q, k, v = split/reshape to [nh, N, hd]
    attn = softmax((q @ k.T) * hd**-0.5)
    out  = (attn @ v) merged heads       # [N, C]
    y    = out @ w_proj.T + b_proj

Sharding: data-parallel over batch B=16 across 8 NeuronCores (2 batches/core).
No collectives.

Device-side design (per core, per batch), v2:
  * All matmul operands are bf16 (PE rate 1.0 cycles/row, same as fp32r, but
    half the DMA/SBUF footprint; PSUM accumulation stays fp32). Overall
    rel-err ~1e-3 vs the 2e-2 budget.
  * Q^T/K^T are projected DENSELY ([1536, N] as 12 m=128 chunks — 25% fewer
    PE rows than per-head m=96 tiles), then per-head [96, N] views are
    either direct slices of the dense buffer (offset-0 heads 0 and 4; the
    birverifier only allows aligned partition blocks) or partition-shifting
    SBUF->SBUF DMA repacks (the other six heads).
  * V in natural [token, nh*(hd+1)] layout with an interleaved ones-column
    per head; scores are computed transposed (S^T[k, q]), exp runs on
    ScalarE with the 1/sqrt(hd) scale fused, row-sums fall out of the
    P~ @ [V | 1] matmul for free.
  * Softmax normalization: one 97-row PSUM->SBUF copy (frees the
    accumulator), DVE reciprocal that moves Z from partition 96 to 0,
    GPSIMD partition_broadcast (replicates absolute partition 0 — no DRAM
    bounce), DVE multiply into a bf16 tile, DMA-repacked into attn^T.
  * A PE warmup on a zeroed tile covers the initial DMA wait so the
    p-state ramp completes before real matmuls start (gaps reset the PE
    clock to 0.65-1.2 GHz for ~3us).
  * Output projection contracts attn^T against w_proj.T in 6 dense
    128-chunks; y is DMA'd out in 384-column halves as soon as each
    PSUM->SBUF copy lands.
  * Dense QK chunk projections and next-head repacks are interleaved into
    the previous head's attention kc-loop to keep the PE fed, and score
    matmuls are emitted one kc ahead of PV so the in-order PE sequencer
    always has work queued while each exp completes.
"""

import os
import sys

sys.path.insert(0, "/opt/trn_rl_repo")

# The Bass->PJRT execution path needs jax to discover the axon-tunneled
# NeuronCores; a stray JAX_PLATFORMS=cpu (e.g. set for a jax reference run)
# would hide them. Only effective if jax hasn't been imported yet.
if "jax" not in sys.modules and "axon" not in os.environ.get("JAX_PLATFORMS", "axon"):
    os.environ.pop("JAX_PLATFORMS", None)

import numpy as np
import ml_dtypes

import concourse.bass as bass
import concourse.mybir as mybir
import concourse.tile as tile
from concourse import bacc
from concourse.bass_utils import run_bass_kernel_spmd

# Problem constants (hardcoded per the task contract).
B, N, C = 16, 1024, 768
NH, HD = 8, 96
NCORES = 8
BPC = B // NCORES  # batches per core = 2
CC = C // 128  # contraction chunks of 128 = 6
KC = N // 128  # key chunks per batch = 8
QH = N // 512  # query halves = 2
TOKC = N // 128  # token chunks for V projection = 8
QC = N // 128  # query chunks for output projection = 8
HDA = HD + 1  # head dim + ones column = 97
VW = NH * HDA  # augmented V width = 776
VHALF = VW // 2  # symmetric V-projection split = 388
QKCH = 12  # dense Q^T/K^T chunks of 128 rows
SCALE = float(HD) ** -0.5

F32 = mybir.dt.float32
BF16 = mybir.dt.bfloat16
NP_BF16 = ml_dtypes.bfloat16

_BUILD_CACHE = {}


def _head_geom(h):
    """Dense-layout geometry of head h's 96 rows: (chunk, part_offset,
    len_in_first_chunk, aligned)."""
    r0 = HD * h
    c0, p0 = divmod(r0, 128)
    len1 = min(HD, 128 - p0)
    # engine APs must be aligned blocks (from partition 32 only 32 partitions
    # are addressable, etc.) -> direct matmul slices only for offset-0 heads
    return c0, p0, len1, p0 == 0


def _build(qk_bias: bool, p_bias: bool):
    """Build + compile the single-core Bass program (shared SPMD across cores)."""
    knobs = tuple(
        int(os.environ.get(k, d))
        for k, d in (
            ("QK_DIRECT", 1),   # direct qkd slices for offset-0 heads
            ("NORM_DIRECT", 0), # 1: normalize straight out of PSUM
            ("PT_BUFS", 3),
            ("QKT_BUFS", 2),
            ("SP_BUFS", 2),
            ("OP_BUFS", 2),
            ("MP_BUFS", 2),
            ("OUT_BUFS", 3),
            ("ON_BUFS", 3),
            ("RB_BUFS", 2),
        )
    )
    key = (qk_bias, p_bias, knobs)
    if key in _BUILD_CACHE:
        return _BUILD_CACHE[key]
    qk_direct, norm_direct, ptb, qktb, spb, opb, mpb, outb, onb, rbb = knobs

    nc = bacc.Bacc("TRN2", target_bir_lowering=False, debug=False)

    xT_d = nc.dram_tensor("xT", [C, BPC * N], BF16, kind="ExternalInput")
    wqk_d = nc.dram_tensor("wqk", [C, 2 * C], BF16, kind="ExternalInput")
    wv_d = nc.dram_tensor("wv", [C, VW], BF16, kind="ExternalInput")
    wp_d = nc.dram_tensor("wp", [C, C], BF16, kind="ExternalInput")
    bvaug_d = nc.dram_tensor("bvaug", [1, VW], BF16, kind="ExternalInput")
    ones_d = nc.dram_tensor("ones", [1, 128], BF16, kind="ExternalInput")
    vones_d = nc.dram_tensor("vones", [128, TOKC, NH], BF16, kind="ExternalInput")
    if qk_bias:
        bqk_d = nc.dram_tensor("bqk", [128, QKCH], F32, kind="ExternalInput")
    if p_bias:
        bp_d = nc.dram_tensor("bp", [1, C], BF16, kind="ExternalInput")
    y_d = nc.dram_tensor("y", [BPC, N, C], F32, kind="ExternalOutput")

    xT_re = xT_d.rearrange("(o p) t -> p o t", p=128)
    wqk_re = wqk_d.rearrange("(o p) f -> p o f", p=128)
    wv_re = wv_d.rearrange("(o p) f -> p o f", p=128)
    wp_re = wp_d.rearrange("(o p) f -> p o f", p=128)

    EXP = mybir.ActivationFunctionType.Exp
    COPY = mybir.ActivationFunctionType.Copy

    # chunk-pair projected during head h's kc-loop (for heads h+1..), and
    # head repacked at kc=4 of head h's loop.
    PROJ_AT = {0: (1, 7), 1: (2, 8), 3: (3, 9), 4: (4, 10), 5: (5, 11)}
    REPACK_AT = {0: 1, 1: 2, 2: 3, 3: 4, 4: 5, 5: 6, 6: 7}

    with tile.TileContext(nc) as tc:
        with (
            tc.tile_pool(name="wpool", bufs=1) as wpool,
            tc.tile_pool(name="xpool", bufs=2) as xpool,
            tc.tile_pool(name="qkd_pool", bufs=1) as qkd_pool,
            tc.tile_pool(name="qkt_pool", bufs=qktb) as qkt_pool,
            tc.tile_pool(name="vpool", bufs=2) as vpool,
            tc.tile_pool(name="pt_pool", bufs=ptb) as pt_pool,
            tc.tile_pool(name="attn_pool", bufs=2) as attn_pool,
            tc.tile_pool(name="rb_pool", bufs=rbb) as rb_pool,
            tc.tile_pool(name="on_pool", bufs=onb) as on_pool,
            tc.tile_pool(name="out_pool", bufs=outb) as out_pool,
            tc.tile_pool(name="spsum", bufs=spb, space="PSUM") as spsum,
            tc.tile_pool(name="opsum_pool", bufs=opb, space="PSUM") as opsum_pool,
            tc.tile_pool(name="mpsum", bufs=mpb, space="PSUM") as mpsum,
        ):
            # --- resident weights/constants ---
            # Startup-ordered on the sync queue: the first V-projection tile
            # needs x quarter 0 + wv's low half; c0/c6 QK chunks come next.
            wv_sb = wpool.tile([128, CC, VW], BF16, tag="wv")
            wqk_sb = wpool.tile([128, CC, 2 * C], BF16, tag="wqk")
            wp_sb = wpool.tile([128, CC, C], BF16, tag="wp")
            bvaug_sb = wpool.tile([1, VW], BF16, tag="bvaug")
            ones_sb = wpool.tile([1, 128], BF16, tag="ones")
            if qk_bias:
                bqk_sb = wpool.tile([128, QKCH], F32, tag="bqk")
            if p_bias:
                bp_sb = wpool.tile([1, C], BF16, tag="bp")

            def emit_weight_dmas(stage):
                if stage == 0:  # x q0 is emitted first by the caller
                    nc.sync.dma_start(wv_sb[:, :, 0:VHALF], wv_re[:, :, 0:VHALF])
                elif stage == 1:  # c0 + c6 QK chunks, V high half
                    nc.sync.dma_start(
                        wqk_sb[:, :, 0:128], wqk_re[:, :, 0:128]
                    )
                    nc.sync.dma_start(
                        wqk_sb[:, :, C : C + 128], wqk_re[:, :, C : C + 128]
                    )
                    nc.sync.dma_start(wv_sb[:, :, VHALF:VW], wv_re[:, :, VHALF:VW])
                    nc.scalar.dma_start(bvaug_sb[:], bvaug_d[:])
                    nc.scalar.dma_start(ones_sb[:], ones_d[:])
                    if qk_bias:
                        nc.scalar.dma_start(bqk_sb[:], bqk_d[:])
                    if p_bias:
                        nc.scalar.dma_start(bp_sb[:], bp_d[:])
                elif stage == 2:  # rest of wqk
                    nc.sync.dma_start(wqk_sb[:, :, 128:C], wqk_re[:, :, 128:C])
                    nc.sync.dma_start(
                        wqk_sb[:, :, C + 128 : 2 * C], wqk_re[:, :, C + 128 : 2 * C]
                    )
                elif stage == 3:  # wp (first used by proj(0), much later)
                    nc.sync.dma_start(wp_sb[:], wp_re[:])

            def emit_x(b, first=0):
                xTb = xpool.tile([128, CC, N], BF16, tag="xTb", name="xTb")
                for xh in range(first, 4):
                    nc.sync.dma_start(
                        xTb[:, :, xh * (N // 4) : (xh + 1) * (N // 4)],
                        xT_re[:, :, b * N + xh * (N // 4) : b * N + (xh + 1) * (N // 4)],
                    )
                return xTb

            def emit_warmup():
                """Dummy matmuls on a zeroed tile while the first input DMAs
                stream in: keeps the PE p-state ramp running so real work
                starts at full clock. PSUM results are never read."""
                wz = wpool.tile([128, 512], BF16, tag="warmze")
                nc.gpsimd.memset(wz[:], 0)
                for i in range(13):
                    wps = mpsum.tile([128, 512], F32, tag="mpsum", name="warm")
                    nc.tensor.matmul(
                        wps[:], wz[:, 0:128], wz[:, :], start=True, stop=True
                    )

            def v_group(v_sb, xTb, t, half, dve_copy=False):
                """One V-projection group: token-chunk t, column half."""
                lo, hi = (0, VHALF) if half == 0 else (VHALF, VW)
                v_bias = bool(qk_bias)  # b_qkv nonzero => v bias nonzero path
                vps = mpsum.tile([128, 512], F32, tag="mpsum", name="vps")
                w = hi - lo
                for cc in range(CC):
                    nc.tensor.matmul(
                        vps[:, :w],
                        xTb[:, cc, t * 128 : (t + 1) * 128],
                        wv_sb[:, cc, lo:hi],
                        start=(cc == 0),
                        stop=(cc == CC - 1 and not v_bias),
                    )
                if v_bias:
                    nc.tensor.matmul(
                        vps[:, :w],
                        ones_sb[:, :],
                        bvaug_sb[:, lo:hi],
                        start=False,
                        stop=True,
                    )
                if dve_copy and not v_bias:
                    nc.vector.tensor_copy(v_sb[:, t, lo:hi], vps[:, :w])
                else:
                    nc.scalar.activation(v_sb[:, t, lo:hi], vps[:, :w], COPY)
                if not v_bias and half == 1:
                    # fill this token-chunk's ones-columns now (a whole-tile
                    # fill would stall head 0's PV on the entire projection)
                    nc.gpsimd.dma_start(
                        v_sb.rearrange("p t (h a) -> p t h a", a=HDA)[
                            :, t, :, HD
                        ],
                        vones_d[:, t, :],
                    )

            def emit_vproj(b, xTb):
                """Project V for batch b, low halves first (the first 8
                groups depend only on wv's low half)."""
                v_sb = vpool.tile([128, TOKC, VW], BF16, tag="v", name="v_sb")
                for half in range(2):
                    for t in range(TOKC):
                        v_group(v_sb, xTb, t, half)
                return v_sb

            def emit_qkgroup(b, xTb, qkd, c, qh):
                """Dense Q^T/K^T chunk c (of 12), query-half qh."""
                qps = mpsum.tile([128, 512], F32, tag="mpsum", name="qps")
                for cc in range(CC):
                    nc.tensor.matmul(
                        qps[:, :],
                        wqk_sb[:, cc, c * 128 : (c + 1) * 128],
                        xTb[:, cc, qh * 512 : (qh + 1) * 512],
                        start=(cc == 0),
                        stop=(cc == CC - 1),
                    )
                dst = qkd[:, c, qh * 512 : (qh + 1) * 512]
                if qk_bias:
                    nc.scalar.activation(
                        dst, qps[:, :], COPY, bias=bqk_sb[:, c : c + 1]
                    )
                else:
                    nc.vector.tensor_copy(dst, qps[:, :])

            def emit_head_view(h, qkd):
                """Per-head [96, N] Q^T/K^T accessors over the dense buffer:
                direct partition-offset slices for aligned heads, DMA repack
                otherwise. Returns (q_ap, k_ap) closures over a column slice."""
                c0, p0, len1, aligned = _head_geom(h)
                if aligned and qk_direct:
                    def q_ap(sl, c0=c0, p0=p0):
                        return qkd[p0 : p0 + HD, c0, sl]

                    def k_ap(sl, c0=c0, p0=p0):
                        return qkd[p0 : p0 + HD, CC + c0, sl]

                    return q_ap, k_ap
                qkt = qkt_pool.tile([128, 2, N], BF16, tag="qkt", name="qkt")
                for f, cbase in ((0, 0), (1, CC)):
                    nc.gpsimd.dma_start(
                        qkt[0:len1, f, :], qkd[p0 : p0 + len1, cbase + c0, :]
                    )
                    if len1 < HD:
                        nc.gpsimd.dma_start(
                            qkt[len1:HD, f, :], qkd[0 : HD - len1, cbase + c0 + 1, :]
                        )

                def q_ap(sl, qkt=qkt):
                    return qkt[0:HD, 0, sl]

                def k_ap(sl, qkt=qkt):
                    return qkt[0:HD, 1, sl]

                return q_ap, k_ap

            def emit_heads(b, xTb, qkd, v_sb, inject=None):
                # densely packed attn^T [C, N]: head h occupies rows
                # HD*h .. HD*h+HD; every row is written (no junk partitions).
                attnT = attn_pool.tile([128, CC, N], BF16, tag="attnT", name="attnT")

                views = [None] * NH
                views[0] = emit_head_view(0, qkd)

                def emit_S(view, kc):
                    q_ap, k_ap = view
                    st = spsum.tile([128, QH, 512], F32, tag="spsum", name="st")
                    for qh in range(QH):
                        nc.tensor.matmul(
                            st[:, qh, :],
                            k_ap(slice(kc * 128, (kc + 1) * 128)),
                            q_ap(slice(qh * 512, (qh + 1) * 512)),
                            start=True,
                            stop=True,
                        )
                    return st

                # scores one kc ahead of PV in emission order (crossing head
                # boundaries): the in-order PE sequencer then always has the
                # next S queued while each exp completes
                st_cur = emit_S(views[0], 0)
                for h in range(NH):
                    q_ap, k_ap = views[h]
                    proj_next = PROJ_AT.get(h)
                    repack_next = REPACK_AT.get(h)

                    # --- attention: S^T blocks, exp, P~ @ [V | 1] ---
                    ops = [
                        opsum_pool.tile([128, 512], F32, tag="opsum", name="ops")
                        for _ in range(QH)
                    ]
                    for kc in range(KC):
                        st_next = emit_S(views[h], kc + 1) if kc + 1 < KC else None
                        pt = pt_pool.tile([128, QH, 512], BF16, tag="pt", name="pt")
                        nc.scalar.activation(pt[:], st_cur[:], EXP, scale=SCALE)
                        for qh in range(QH):
                            nc.tensor.matmul(
                                ops[qh][:HDA, :],
                                v_sb[:, kc, HDA * h : HDA * (h + 1)],
                                pt[:, qh, :],
                                start=(kc == 0),
                                stop=(kc == KC - 1),
                            )
                        # interleave next heads' dense projection + repack
                        if proj_next is not None and kc < 4:
                            ca, cb = proj_next
                            c = ca if kc < 2 else cb
                            emit_qkgroup(b, xTb, qkd, c, kc % 2)
                        if repack_next is not None and kc == 4:
                            views[h + 1] = emit_head_view(h + 1, qkd)
                        if inject is not None:
                            for fn in inject.get((h, kc), ()):
                                fn()
                        if kc == KC - 1 and h + 1 < NH:
                            st_next = emit_S(views[h + 1], 0)
                        st_cur = st_next

                    # --- normalize O^T by 1/rowsum and repack into attn^T ---
                    r0 = HD * h
                    cc0, p0 = divmod(r0, 128)
                    len1 = min(HD, 128 - p0)
                    for qh in range(QH):
                        qs = slice(qh * 512, (qh + 1) * 512)
                        rb = rb_pool.tile([1, 512], F32, tag="rb", name="rb")
                        rbB = rb_pool.tile([128, 512], F32, tag="rbB", name="rbB")
                        oN = on_pool.tile([128, 512], BF16, tag="oN", name="oN")
                        # single 97-row copy frees the PSUM accumulator ASAP;
                        # the reciprocal moves Z from partition 96 to 0 (DVE
                        # can cross partitions between aligned starts), and
                        # GPSIMD partition_broadcast replicates from absolute
                        # partition 0 (its HW semantics).
                        ot = on_pool.tile([128, 512], F32, tag="ot", name="ot")
                        nc.vector.tensor_copy(ot[:HDA, :], ops[qh][:HDA, :])
                        nc.vector.reciprocal(rb[0:1, :], ot[HD : HD + 1, :])
                        nc.gpsimd.partition_broadcast(rbB[0:HD, :], rb[0:1, :])
                        nc.vector.tensor_tensor(
                            oN[:HD, :], ot[:HD, :], rbB[:HD, :],
                            mybir.AluOpType.mult,
                        )
                        nc.gpsimd.dma_start(
                            attnT[p0 : p0 + len1, cc0, qs], oN[:len1, :]
                        )
                        if len1 < HD:
                            nc.gpsimd.dma_start(
                                attnT[0 : HD - len1, cc0 + 1, qs],
                                oN[len1:HD, :],
                            )

                return attnT

            def proj_group(b, attnT, out_sb, qc, n, fine=False, dve_copy=False):
                """One output-projection group: query-chunk qc, half n."""
                gw = C // 2
                pps = mpsum.tile([128, 512], F32, tag="mpsum", name="pps")
                ns = slice(n * gw, (n + 1) * gw)
                for cc in range(CC):
                    nc.tensor.matmul(
                        pps[:, :gw],
                        attnT[:, cc, qc * 128 : (qc + 1) * 128],
                        wp_sb[:, cc, ns],
                        start=(cc == 0),
                        stop=(cc == CC - 1 and not p_bias),
                    )
                if p_bias:
                    nc.tensor.matmul(
                        pps[:, :gw],
                        ones_sb[:, :],
                        bp_sb[:, ns],
                        start=False,
                        stop=True,
                    )
                if fine and n == 1:
                    # epilogue: two 192-col copy+DMA chains on parallel
                    # queues so the last piece's latency is minimal
                    for pp, q in ((0, nc.sync), (1, nc.scalar)):
                        ps = slice(n * gw + pp * 192, n * gw + (pp + 1) * 192)
                        nc.scalar.activation(
                            out_sb[:, ps], pps[:, pp * 192 : (pp + 1) * 192], COPY
                        )
                        q.dma_start(
                            y_d[b, qc * 128 : (qc + 1) * 128, ps], out_sb[:, ps]
                        )
                    return
                if dve_copy and not p_bias:
                    nc.vector.tensor_copy(out_sb[:, ns], pps[:, :gw])
                else:
                    nc.scalar.activation(out_sb[:, ns], pps[:, :gw], COPY)
                nc.sync.dma_start(
                    y_d[b, qc * 128 : (qc + 1) * 128, ns], out_sb[:, ns]
                )

            def emit_proj(b, attnT, last=False):
                for qc in range(QC):
                    out_sb = out_pool.tile([128, C], F32, tag="out", name="out_sb")
                    fine = last and qc == QC - 1
                    for n in range(2):
                        proj_group(b, attnT, out_sb, qc, n, fine=fine)

            def emit_qk_head0(b, xTb, qkd):
                # c0/c6 ordered so S(kc0,qh0)'s operands land first
                emit_qkgroup(b, xTb, qkd, 0, 0)
                emit_qkgroup(b, xTb, qkd, CC, 0)
                emit_qkgroup(b, xTb, qkd, 0, 1)
                emit_qkgroup(b, xTb, qkd, CC, 1)

            # Emission (≈ static engine) order: batch 1's V projection and
            # head-0 QK chunks fill the PE while batch 0's normalize tail
            # drains; proj(0) is deferred past heads(1) likewise.
            xTb0 = xpool.tile([128, CC, N], BF16, tag="xTb", name="xTb")
            nc.sync.dma_start(
                xTb0[:, :, 0 : N // 4], xT_re[:, :, 0 : N // 4]
            )
            emit_weight_dmas(0)
            for xh in range(1, 4):
                nc.sync.dma_start(
                    xTb0[:, :, xh * (N // 4) : (xh + 1) * (N // 4)],
                    xT_re[:, :, xh * (N // 4) : (xh + 1) * (N // 4)],
                )
            emit_weight_dmas(1)
            emit_warmup()
            qkd0 = qkd_pool.tile([128, QKCH, N], BF16, tag="qkd", name="qkd")
            v0 = emit_vproj(0, xTb0)
            emit_weight_dmas(2)
            emit_qk_head0(0, xTb0, qkd0)
            at0 = emit_heads(0, xTb0, qkd0, v0)
            xTb1 = emit_x(1)
            v1 = emit_vproj(1, xTb1)
            qkd1 = qkd_pool.tile([128, QKCH, N], BF16, tag="qkd", name="qkd")
            emit_weight_dmas(3)
            emit_qk_head0(1, xTb1, qkd1)
            at1 = emit_heads(1, xTb1, qkd1, v1)
            emit_proj(0, at0)
            emit_proj(1, at1, last=True)

    nc.compile()
    _BUILD_CACHE[key] = nc
    return nc


def _prep_shared(w_qkv, b_qkv, w_proj, b_proj):
    """Host-side weight rearrangement shared by all cores."""
    w_qkv = np.ascontiguousarray(w_qkv, dtype=np.float32)
    w_proj = np.ascontiguousarray(w_proj, dtype=np.float32)
    b_qkv = np.asarray(b_qkv, dtype=np.float32)
    b_proj = np.asarray(b_proj, dtype=np.float32)

    # wqk: [C, 2C] dense — column f = w_qkv row f (Q rows then K rows)
    wqk_arr = np.ascontiguousarray(w_qkv[: 2 * C].T.astype(NP_BF16))

    # wv: [C, NH*(HD+1)] with a zero ones-column slot per head
    wv = w_qkv[2 * C :].reshape(NH, HD, C)  # [h, j, c]
    wv_aug = np.zeros((C, NH, HDA), dtype=np.float32)
    wv_aug[:, :, :HD] = np.transpose(wv, (2, 0, 1))  # col HD = ones slot
    wv_aug = np.ascontiguousarray(wv_aug.reshape(C, VW).astype(NP_BF16))

    # wp: plain transpose [c_in, c_out]
    wp_t = np.ascontiguousarray(w_proj.T.astype(NP_BF16))

    # bvaug: v-bias interleaved with 1.0 at each head's ones-column
    bvaug = np.zeros((1, NH, HDA), dtype=np.float32)
    bvaug[0, :, :HD] = b_qkv[2 * C :].reshape(NH, HD)
    bvaug[0, :, HD] = 1.0
    bvaug = bvaug.reshape(1, VW).astype(NP_BF16)

    ones = np.ones((1, 128), dtype=NP_BF16)
    vones = np.ones((128, TOKC, NH), dtype=NP_BF16)

    qk_bias = bool(np.any(b_qkv[: 2 * C] != 0.0))
    p_bias = bool(np.any(b_proj != 0.0))
    extra = {}
    if qk_bias:
        # dense per-partition bias: chunk c partition p = b_qkv[128c + p]
        extra["bqk"] = np.ascontiguousarray(
            b_qkv[: 2 * C].reshape(QKCH, 128).T
        )
    if p_bias:
        extra["bp"] = np.ascontiguousarray(b_proj.reshape(1, C).astype(NP_BF16))

    return wqk_arr, wv_aug, wp_t, bvaug, ones, vones, qk_bias, p_bias, extra


def kernel(x, w_qkv, b_qkv, w_proj, b_proj, H=32, W=32):
    x = np.asarray(x, dtype=np.float32)
    assert x.shape == (B, N, C), x.shape
    assert int(H) * int(W) == N

    wqk_arr, wv_aug, wp_t, bvaug, ones, vones, qk_bias, p_bias, extra = _prep_shared(
        w_qkv, b_qkv, w_proj, b_proj
    )
    nc = _build(qk_bias, p_bias)

    in_maps = []
    for c in range(NCORES):
        xc = x[BPC * c : BPC * (c + 1)].reshape(BPC * N, C)
        xT = np.ascontiguousarray(xc.T.astype(NP_BF16))  # [C, BPC*N]
        m = {
            "xT": xT,
            "wqk": wqk_arr,
            "wv": wv_aug,
            "wp": wp_t,
            "bvaug": bvaug,
            "ones": ones,
            "vones": vones,
        }
        m.update(extra)
        in_maps.append(m)

    trace = os.environ.get("KERNEL_TRACE") == "1"
    res = run_bass_kernel_spmd(
        nc, in_maps, core_ids=list(range(NCORES)), trace=trace
    )
    if trace:
        kernel.last_results = res
        print("exec_time_ns:", res.exec_time_ns, "mean:", res.mean_exec_time_ns)
        if res.instructions_and_trace:
            print("trace:", res.instructions_and_trace[1])

    out = np.empty((B, N, C), dtype=np.float32)
    for c in range(NCORES):
        out[BPC * c : BPC * (c + 1)] = res.results[c]["y"]
    return out


if __name__ == "__main__":
    rng = np.random.default_rng(0)
    x = rng.standard_normal((B, N, C), dtype=np.float32)
    w_qkv = rng.standard_normal((3 * C, C), dtype=np.float32) / np.sqrt(C)
    b_qkv = np.zeros(3 * C, np.float32)
    w_proj = rng.standard_normal((C, C), dtype=np.float32) / np.sqrt(C)
    b_proj = np.zeros(C, np.float32)
    y = kernel(x, w_qkv, b_qkv, w_proj, b_proj)
    print("out", y.shape, y.dtype, float(np.abs(y).mean()))


# revision 33
# speedup vs baseline: 1.0007x; 1.0007x over previous
"""Trainium2 Bass kernel for multi-head global attention (the
"DeformableAttention" module whose relative-position-bias path is inactive).

Reference computation (per batch b):
    qkv = x @ w_qkv.T + b_qkv            # [N, 3C]
    q, k, v = split/reshape to [nh, N, hd]
    attn = softmax((q @ k.T) * hd**-0.5)
    out  = (attn @ v) merged heads       # [N, C]
    y    = out @ w_proj.T + b_proj

Sharding: data-parallel over batch B=16 across 8 NeuronCores (2 batches/core).
No collectives.

Device-side design (per core, per batch), v2:
  * All matmul operands are bf16 (PE rate 1.0 cycles/row, same as fp32r, but
    half the DMA/SBUF footprint; PSUM accumulation stays fp32). Overall
    rel-err ~1e-3 vs the 2e-2 budget.
  * Q^T/K^T are projected DENSELY ([1536, N] as 12 m=128 chunks — 25% fewer
    PE rows than per-head m=96 tiles), then per-head [96, N] views are
    either direct slices of the dense buffer (offset-0 heads 0 and 4; the
    birverifier only allows aligned partition blocks) or partition-shifting
    SBUF->SBUF DMA repacks (the other six heads).
  * V in natural [token, nh*(hd+1)] layout with an interleaved ones-column
    per head; scores are computed transposed (S^T[k, q]), exp runs on
    ScalarE with the 1/sqrt(hd) scale fused, row-sums fall out of the
    P~ @ [V | 1] matmul for free.
  * Softmax normalization: one 97-row PSUM->SBUF copy (frees the
    accumulator), DVE reciprocal that moves Z from partition 96 to 0,
    GPSIMD partition_broadcast (replicates absolute partition 0 — no DRAM
    bounce), DVE multiply into a bf16 tile, DMA-repacked into attn^T.
  * A PE warmup on a zeroed tile covers the initial DMA wait so the
    p-state ramp completes before real matmuls start (gaps reset the PE
    clock to 0.65-1.2 GHz for ~3us).
  * Output projection contracts attn^T against w_proj.T in 6 dense
    128-chunks; y is DMA'd out in 384-column halves as soon as each
    PSUM->SBUF copy lands.
  * Dense QK chunk projections and next-head repacks are interleaved into
    the previous head's attention kc-loop to keep the PE fed, and score
    matmuls are emitted one kc ahead of PV so the in-order PE sequencer
    always has work queued while each exp completes.
"""

import os
import sys

sys.path.insert(0, "/opt/trn_rl_repo")

# The Bass->PJRT execution path needs jax to discover the axon-tunneled
# NeuronCores; a stray JAX_PLATFORMS=cpu (e.g. set for a jax reference run)
# would hide them. Only effective if jax hasn't been imported yet.
if "jax" not in sys.modules and "axon" not in os.environ.get("JAX_PLATFORMS", "axon"):
    os.environ.pop("JAX_PLATFORMS", None)

import numpy as np
import ml_dtypes

import concourse.bass as bass
import concourse.mybir as mybir
import concourse.tile as tile
from concourse import bacc
from concourse.bass_utils import run_bass_kernel_spmd

# Problem constants (hardcoded per the task contract).
B, N, C = 16, 1024, 768
NH, HD = 8, 96
NCORES = 8
BPC = B // NCORES  # batches per core = 2
CC = C // 128  # contraction chunks of 128 = 6
KC = N // 128  # key chunks per batch = 8
QH = N // 512  # query halves = 2
TOKC = N // 128  # token chunks for V projection = 8
QC = N // 128  # query chunks for output projection = 8
HDA = HD + 1  # head dim + ones column = 97
VW = NH * HDA  # augmented V width = 776
VHALF = VW // 2  # symmetric V-projection split = 388
QKCH = 12  # dense Q^T/K^T chunks of 128 rows
SCALE = float(HD) ** -0.5

F32 = mybir.dt.float32
BF16 = mybir.dt.bfloat16
NP_BF16 = ml_dtypes.bfloat16

_BUILD_CACHE = {}


def _head_geom(h):
    """Dense-layout geometry of head h's 96 rows: (chunk, part_offset,
    len_in_first_chunk, aligned)."""
    r0 = HD * h
    c0, p0 = divmod(r0, 128)
    len1 = min(HD, 128 - p0)
    # engine APs must be aligned blocks (from partition 32 only 32 partitions
    # are addressable, etc.) -> direct matmul slices only for offset-0 heads
    return c0, p0, len1, p0 == 0


def _build(qk_bias: bool, p_bias: bool):
    """Build + compile the single-core Bass program (shared SPMD across cores)."""
    knobs = tuple(
        int(os.environ.get(k, d))
        for k, d in (
            ("QK_DIRECT", 1),   # direct qkd slices for offset-0 heads
            ("NORM_DIRECT", 0), # 1: normalize straight out of PSUM
            ("PT_BUFS", 3),
            ("QKT_BUFS", 2),
            ("SP_BUFS", 2),
            ("OP_BUFS", 2),
            ("MP_BUFS", 2),
            ("OUT_BUFS", 3),
            ("ON_BUFS", 3),
            ("RB_BUFS", 2),
        )
    )
    key = (qk_bias, p_bias, knobs)
    if key in _BUILD_CACHE:
        return _BUILD_CACHE[key]
    qk_direct, norm_direct, ptb, qktb, spb, opb, mpb, outb, onb, rbb = knobs

    nc = bacc.Bacc("TRN2", target_bir_lowering=False, debug=False)

    xT_d = nc.dram_tensor("xT", [C, BPC * N], BF16, kind="ExternalInput")
    wqk_d = nc.dram_tensor("wqk", [C, 2 * C], BF16, kind="ExternalInput")
    wv_d = nc.dram_tensor("wv", [C, VW], BF16, kind="ExternalInput")
    wp_d = nc.dram_tensor("wp", [C, C], BF16, kind="ExternalInput")
    bvaug_d = nc.dram_tensor("bvaug", [1, VW], BF16, kind="ExternalInput")
    ones_d = nc.dram_tensor("ones", [1, 128], BF16, kind="ExternalInput")
    vones_d = nc.dram_tensor("vones", [128, TOKC, NH], BF16, kind="ExternalInput")
    if qk_bias:
        bqk_d = nc.dram_tensor("bqk", [128, QKCH], F32, kind="ExternalInput")
    if p_bias:
        bp_d = nc.dram_tensor("bp", [1, C], BF16, kind="ExternalInput")
    y_d = nc.dram_tensor("y", [BPC, N, C], F32, kind="ExternalOutput")

    xT_re = xT_d.rearrange("(o p) t -> p o t", p=128)
    wqk_re = wqk_d.rearrange("(o p) f -> p o f", p=128)
    wv_re = wv_d.rearrange("(o p) f -> p o f", p=128)
    wp_re = wp_d.rearrange("(o p) f -> p o f", p=128)

    EXP = mybir.ActivationFunctionType.Exp
    COPY = mybir.ActivationFunctionType.Copy

    # chunk-pair projected during head h's kc-loop (for heads h+1..), and
    # head repacked at kc=4 of head h's loop.
    PROJ_AT = {0: (1, 7), 1: (2, 8), 3: (3, 9), 4: (4, 10), 5: (5, 11)}
    REPACK_AT = {0: 1, 1: 2, 2: 3, 3: 4, 4: 5, 5: 6, 6: 7}

    with tile.TileContext(nc) as tc:
        with (
            tc.tile_pool(name="wpool", bufs=1) as wpool,
            tc.tile_pool(name="xpool", bufs=2) as xpool,
            tc.tile_pool(name="qkd_pool", bufs=1) as qkd_pool,
            tc.tile_pool(name="qkt_pool", bufs=qktb) as qkt_pool,
            tc.tile_pool(name="vpool", bufs=2) as vpool,
            tc.tile_pool(name="pt_pool", bufs=ptb) as pt_pool,
            tc.tile_pool(name="attn_pool", bufs=2) as attn_pool,
            tc.tile_pool(name="rb_pool", bufs=rbb) as rb_pool,
            tc.tile_pool(name="on_pool", bufs=onb) as on_pool,
            tc.tile_pool(name="out_pool", bufs=outb) as out_pool,
            tc.tile_pool(name="spsum", bufs=spb, space="PSUM") as spsum,
            tc.tile_pool(name="opsum_pool", bufs=opb, space="PSUM") as opsum_pool,
            tc.tile_pool(name="mpsum", bufs=mpb, space="PSUM") as mpsum,
        ):
            # --- resident weights/constants ---
            # Startup-ordered on the sync queue: the first V-projection tile
            # needs x quarter 0 + wv's low half; c0/c6 QK chunks come next.
            wv_sb = wpool.tile([128, CC, VW], BF16, tag="wv")
            wqk_sb = wpool.tile([128, CC, 2 * C], BF16, tag="wqk")
            wp_sb = wpool.tile([128, CC, C], BF16, tag="wp")
            bvaug_sb = wpool.tile([1, VW], BF16, tag="bvaug")
            ones_sb = wpool.tile([1, 128], BF16, tag="ones")
            if qk_bias:
                bqk_sb = wpool.tile([128, QKCH], F32, tag="bqk")
            if p_bias:
                bp_sb = wpool.tile([1, C], BF16, tag="bp")

            def emit_weight_dmas(stage):
                if stage == 0:  # x q0 is emitted first by the caller;
                    # wv-lo goes via the Act queue so its DGE setup overlaps
                    # x-q0's on the sync queue
                    nc.scalar.dma_start(wv_sb[:, :, 0:VHALF], wv_re[:, :, 0:VHALF])
                elif stage == 1:  # c0 + c6 QK chunks, V high half
                    nc.sync.dma_start(
                        wqk_sb[:, :, 0:128], wqk_re[:, :, 0:128]
                    )
                    nc.sync.dma_start(
                        wqk_sb[:, :, C : C + 128], wqk_re[:, :, C : C + 128]
                    )
                    nc.sync.dma_start(wv_sb[:, :, VHALF:VW], wv_re[:, :, VHALF:VW])
                    nc.scalar.dma_start(bvaug_sb[:], bvaug_d[:])
                    nc.scalar.dma_start(ones_sb[:], ones_d[:])
                    if qk_bias:
                        nc.scalar.dma_start(bqk_sb[:], bqk_d[:])
                    if p_bias:
                        nc.scalar.dma_start(bp_sb[:], bp_d[:])
                elif stage == 2:  # rest of wqk
                    nc.sync.dma_start(wqk_sb[:, :, 128:C], wqk_re[:, :, 128:C])
                    nc.sync.dma_start(
                        wqk_sb[:, :, C + 128 : 2 * C], wqk_re[:, :, C + 128 : 2 * C]
                    )
                elif stage == 3:  # wp (first used by proj(0), much later)
                    nc.sync.dma_start(wp_sb[:], wp_re[:])

            def emit_x(b, first=0):
                xTb = xpool.tile([128, CC, N], BF16, tag="xTb", name="xTb")
                for xh in range(first, 4):
                    nc.sync.dma_start(
                        xTb[:, :, xh * (N // 4) : (xh + 1) * (N // 4)],
                        xT_re[:, :, b * N + xh * (N // 4) : b * N + (xh + 1) * (N // 4)],
                    )
                return xTb

            def emit_warmup():
                """Dummy matmuls on a zeroed tile while the first input DMAs
                stream in: keeps the PE p-state ramp running so real work
                starts at full clock. PSUM results are never read."""
                wz = wpool.tile([128, 512], BF16, tag="warmze")
                nc.gpsimd.memset(wz[:], 0)
                for i in range(12):
                    wps = mpsum.tile([128, 512], F32, tag="mpsum", name="warm")
                    nc.tensor.matmul(
                        wps[:], wz[:, 0:128], wz[:, :], start=True, stop=True
                    )

            def v_group(v_sb, xTb, t, half, dve_copy=False):
                """One V-projection group: token-chunk t, column half."""
                lo, hi = (0, VHALF) if half == 0 else (VHALF, VW)
                v_bias = bool(qk_bias)  # b_qkv nonzero => v bias nonzero path
                vps = mpsum.tile([128, 512], F32, tag="mpsum", name="vps")
                w = hi - lo
                for cc in range(CC):
                    nc.tensor.matmul(
                        vps[:, :w],
                        xTb[:, cc, t * 128 : (t + 1) * 128],
                        wv_sb[:, cc, lo:hi],
                        start=(cc == 0),
                        stop=(cc == CC - 1 and not v_bias),
                    )
                if v_bias:
                    nc.tensor.matmul(
                        vps[:, :w],
                        ones_sb[:, :],
                        bvaug_sb[:, lo:hi],
                        start=False,
                        stop=True,
                    )
                if dve_copy and not v_bias:
                    nc.vector.tensor_copy(v_sb[:, t, lo:hi], vps[:, :w])
                else:
                    nc.scalar.activation(v_sb[:, t, lo:hi], vps[:, :w], COPY)
                if not v_bias and half == 1:
                    # fill this token-chunk's ones-columns now (a whole-tile
                    # fill would stall head 0's PV on the entire projection)
                    nc.gpsimd.dma_start(
                        v_sb.rearrange("p t (h a) -> p t h a", a=HDA)[
                            :, t, :, HD
                        ],
                        vones_d[:, t, :],
                    )

            def emit_vproj(b, xTb):
                """Project V for batch b, low halves first (the first 8
                groups depend only on wv's low half)."""
                v_sb = vpool.tile([128, TOKC, VW], BF16, tag="v", name="v_sb")
                for half in range(2):
                    for t in range(TOKC):
                        v_group(v_sb, xTb, t, half)
                return v_sb

            def emit_qkgroup(b, xTb, qkd, c, qh):
                """Dense Q^T/K^T chunk c (of 12), query-half qh."""
                qps = mpsum.tile([128, 512], F32, tag="mpsum", name="qps")
                for cc in range(CC):
                    nc.tensor.matmul(
                        qps[:, :],
                        wqk_sb[:, cc, c * 128 : (c + 1) * 128],
                        xTb[:, cc, qh * 512 : (qh + 1) * 512],
                        start=(cc == 0),
                        stop=(cc == CC - 1),
                    )
                dst = qkd[:, c, qh * 512 : (qh + 1) * 512]
                if qk_bias:
                    nc.scalar.activation(
                        dst, qps[:, :], COPY, bias=bqk_sb[:, c : c + 1]
                    )
                else:
                    nc.vector.tensor_copy(dst, qps[:, :])

            def emit_head_view(h, qkd):
                """Per-head [96, N] Q^T/K^T accessors over the dense buffer:
                direct partition-offset slices for aligned heads, DMA repack
                otherwise. Returns (q_ap, k_ap) closures over a column slice."""
                c0, p0, len1, aligned = _head_geom(h)
                if aligned and qk_direct:
                    def q_ap(sl, c0=c0, p0=p0):
                        return qkd[p0 : p0 + HD, c0, sl]

                    def k_ap(sl, c0=c0, p0=p0):
                        return qkd[p0 : p0 + HD, CC + c0, sl]

                    return q_ap, k_ap
                qkt = qkt_pool.tile([128, 2, N], BF16, tag="qkt", name="qkt")
                for f, cbase in ((0, 0), (1, CC)):
                    nc.gpsimd.dma_start(
                        qkt[0:len1, f, :], qkd[p0 : p0 + len1, cbase + c0, :]
                    )
                    if len1 < HD:
                        nc.gpsimd.dma_start(
                            qkt[len1:HD, f, :], qkd[0 : HD - len1, cbase + c0 + 1, :]
                        )

                def q_ap(sl, qkt=qkt):
                    return qkt[0:HD, 0, sl]

                def k_ap(sl, qkt=qkt):
                    return qkt[0:HD, 1, sl]

                return q_ap, k_ap

            def emit_heads(b, xTb, qkd, v_sb, inject=None):
                # densely packed attn^T [C, N]: head h occupies rows
                # HD*h .. HD*h+HD; every row is written (no junk partitions).
                attnT = attn_pool.tile([128, CC, N], BF16, tag="attnT", name="attnT")

                views = [None] * NH
                views[0] = emit_head_view(0, qkd)

                def emit_S(view, kc):
                    q_ap, k_ap = view
                    st = spsum.tile([128, QH, 512], F32, tag="spsum", name="st")
                    for qh in range(QH):
                        nc.tensor.matmul(
                            st[:, qh, :],
                            k_ap(slice(kc * 128, (kc + 1) * 128)),
                            q_ap(slice(qh * 512, (qh + 1) * 512)),
                            start=True,
                            stop=True,
                        )
                    return st

                # scores one kc ahead of PV in emission order (crossing head
                # boundaries): the in-order PE sequencer then always has the
                # next S queued while each exp completes
                st_cur = emit_S(views[0], 0)
                for h in range(NH):
                    q_ap, k_ap = views[h]
                    proj_next = PROJ_AT.get(h)
                    repack_next = REPACK_AT.get(h)

                    # --- attention: S^T blocks, exp, P~ @ [V | 1] ---
                    ops = [
                        opsum_pool.tile([128, 512], F32, tag="opsum", name="ops")
                        for _ in range(QH)
                    ]
                    for kc in range(KC):
                        st_next = emit_S(views[h], kc + 1) if kc + 1 < KC else None
                        pt = pt_pool.tile([128, QH, 512], BF16, tag="pt", name="pt")
                        nc.scalar.activation(pt[:], st_cur[:], EXP, scale=SCALE)
                        for qh in range(QH):
                            nc.tensor.matmul(
                                ops[qh][:HDA, :],
                                v_sb[:, kc, HDA * h : HDA * (h + 1)],
                                pt[:, qh, :],
                                start=(kc == 0),
                                stop=(kc == KC - 1),
                            )
                        # interleave next heads' dense projection + repack
                        if proj_next is not None and kc < 4:
                            ca, cb = proj_next
                            c = ca if kc < 2 else cb
                            emit_qkgroup(b, xTb, qkd, c, kc % 2)
                        if repack_next is not None and kc == 4:
                            views[h + 1] = emit_head_view(h + 1, qkd)
                        if inject is not None:
                            for fn in inject.get((h, kc), ()):
                                fn()
                        if kc == KC - 1 and h + 1 < NH:
                            st_next = emit_S(views[h + 1], 0)
                        st_cur = st_next

                    # --- normalize O^T by 1/rowsum and repack into attn^T ---
                    r0 = HD * h
                    cc0, p0 = divmod(r0, 128)
                    len1 = min(HD, 128 - p0)
                    for qh in range(QH):
                        qs = slice(qh * 512, (qh + 1) * 512)
                        rb = rb_pool.tile([1, 512], F32, tag="rb", name="rb")
                        rbB = rb_pool.tile([128, 512], F32, tag="rbB", name="rbB")
                        oN = on_pool.tile([128, 512], BF16, tag="oN", name="oN")
                        # single 97-row copy frees the PSUM accumulator ASAP;
                        # the reciprocal moves Z from partition 96 to 0 (DVE
                        # can cross partitions between aligned starts), and
                        # GPSIMD partition_broadcast replicates from absolute
                        # partition 0 (its HW semantics).
                        ot = on_pool.tile([128, 512], F32, tag="ot", name="ot")
                        nc.vector.tensor_copy(ot[:HDA, :], ops[qh][:HDA, :])
                        nc.vector.reciprocal(rb[0:1, :], ot[HD : HD + 1, :])
                        nc.gpsimd.partition_broadcast(rbB[0:HD, :], rb[0:1, :])
                        nc.vector.tensor_tensor(
                            oN[:HD, :], ot[:HD, :], rbB[:HD, :],
                            mybir.AluOpType.mult,
                        )
                        nc.gpsimd.dma_start(
                            attnT[p0 : p0 + len1, cc0, qs], oN[:len1, :]
                        )
                        if len1 < HD:
                            nc.gpsimd.dma_start(
                                attnT[0 : HD - len1, cc0 + 1, qs],
                                oN[len1:HD, :],
                            )

                return attnT

            def proj_group(b, attnT, out_sb, qc, n, fine=False, dve_copy=False):
                """One output-projection group: query-chunk qc, half n."""
                gw = C // 2
                pps = mpsum.tile([128, 512], F32, tag="mpsum", name="pps")
                ns = slice(n * gw, (n + 1) * gw)
                for cc in range(CC):
                    nc.tensor.matmul(
                        pps[:, :gw],
                        attnT[:, cc, qc * 128 : (qc + 1) * 128],
                        wp_sb[:, cc, ns],
                        start=(cc == 0),
                        stop=(cc == CC - 1 and not p_bias),
                    )
                if p_bias:
                    nc.tensor.matmul(
                        pps[:, :gw],
                        ones_sb[:, :],
                        bp_sb[:, ns],
                        start=False,
                        stop=True,
                    )
                if fine and n == 1:
                    # epilogue: two 192-col copy+DMA chains on parallel
                    # queues so the last piece's latency is minimal
                    for pp, q in ((0, nc.sync), (1, nc.scalar)):
                        ps = slice(n * gw + pp * 192, n * gw + (pp + 1) * 192)
                        nc.scalar.activation(
                            out_sb[:, ps], pps[:, pp * 192 : (pp + 1) * 192], COPY
                        )
                        q.dma_start(
                            y_d[b, qc * 128 : (qc + 1) * 128, ps], out_sb[:, ps]
                        )
                    return
                if dve_copy and not p_bias:
                    nc.vector.tensor_copy(out_sb[:, ns], pps[:, :gw])
                else:
                    nc.scalar.activation(out_sb[:, ns], pps[:, :gw], COPY)
                nc.sync.dma_start(
                    y_d[b, qc * 128 : (qc + 1) * 128, ns], out_sb[:, ns]
                )

            def emit_proj(b, attnT, last=False):
                for qc in range(QC):
                    out_sb = out_pool.tile([128, C], F32, tag="out", name="out_sb")
                    fine = last and qc == QC - 1
                    for n in range(2):
                        proj_group(b, attnT, out_sb, qc, n, fine=fine)

            def emit_qk_head0(b, xTb, qkd):
                # c0/c6 ordered so S(kc0,qh0)'s operands land first
                emit_qkgroup(b, xTb, qkd, 0, 0)
                emit_qkgroup(b, xTb, qkd, CC, 0)
                emit_qkgroup(b, xTb, qkd, 0, 1)
                emit_qkgroup(b, xTb, qkd, CC, 1)

            # Emission (≈ static engine) order: batch 1's V projection and
            # head-0 QK chunks fill the PE while batch 0's normalize tail
            # drains; proj(0) is deferred past heads(1) likewise.
            xTb0 = xpool.tile([128, CC, N], BF16, tag="xTb", name="xTb")
            nc.sync.dma_start(
                xTb0[:, :, 0 : N // 4], xT_re[:, :, 0 : N // 4]
            )
            emit_weight_dmas(0)
            for xh in range(1, 4):
                nc.sync.dma_start(
                    xTb0[:, :, xh * (N // 4) : (xh + 1) * (N // 4)],
                    xT_re[:, :, xh * (N // 4) : (xh + 1) * (N // 4)],
                )
            emit_weight_dmas(1)
            emit_warmup()
            qkd0 = qkd_pool.tile([128, QKCH, N], BF16, tag="qkd", name="qkd")
            v0 = emit_vproj(0, xTb0)
            emit_weight_dmas(2)
            emit_qk_head0(0, xTb0, qkd0)
            at0 = emit_heads(0, xTb0, qkd0, v0)
            xTb1 = emit_x(1)
            v1 = emit_vproj(1, xTb1)
            qkd1 = qkd_pool.tile([128, QKCH, N], BF16, tag="qkd", name="qkd")
            emit_weight_dmas(3)
            emit_qk_head0(1, xTb1, qkd1)
            at1 = emit_heads(1, xTb1, qkd1, v1)
            emit_proj(0, at0)
            emit_proj(1, at1, last=True)

    nc.compile()
    _BUILD_CACHE[key] = nc
    return nc


def _prep_shared(w_qkv, b_qkv, w_proj, b_proj):
    """Host-side weight rearrangement shared by all cores."""
    w_qkv = np.ascontiguousarray(w_qkv, dtype=np.float32)
    w_proj = np.ascontiguousarray(w_proj, dtype=np.float32)
    b_qkv = np.asarray(b_qkv, dtype=np.float32)
    b_proj = np.asarray(b_proj, dtype=np.float32)

    # wqk: [C, 2C] dense — column f = w_qkv row f (Q rows then K rows)
    wqk_arr = np.ascontiguousarray(w_qkv[: 2 * C].T.astype(NP_BF16))

    # wv: [C, NH*(HD+1)] with a zero ones-column slot per head
    wv = w_qkv[2 * C :].reshape(NH, HD, C)  # [h, j, c]
    wv_aug = np.zeros((C, NH, HDA), dtype=np.float32)
    wv_aug[:, :, :HD] = np.transpose(wv, (2, 0, 1))  # col HD = ones slot
    wv_aug = np.ascontiguousarray(wv_aug.reshape(C, VW).astype(NP_BF16))

    # wp: plain transpose [c_in, c_out]
    wp_t = np.ascontiguousarray(w_proj.T.astype(NP_BF16))

    # bvaug: v-bias interleaved with 1.0 at each head's ones-column
    bvaug = np.zeros((1, NH, HDA), dtype=np.float32)
    bvaug[0, :, :HD] = b_qkv[2 * C :].reshape(NH, HD)
    bvaug[0, :, HD] = 1.0
    bvaug = bvaug.reshape(1, VW).astype(NP_BF16)

    ones = np.ones((1, 128), dtype=NP_BF16)
    vones = np.ones((128, TOKC, NH), dtype=NP_BF16)

    qk_bias = bool(np.any(b_qkv[: 2 * C] != 0.0))
    p_bias = bool(np.any(b_proj != 0.0))
    extra = {}
    if qk_bias:
        # dense per-partition bias: chunk c partition p = b_qkv[128c + p]
        extra["bqk"] = np.ascontiguousarray(
            b_qkv[: 2 * C].reshape(QKCH, 128).T
        )
    if p_bias:
        extra["bp"] = np.ascontiguousarray(b_proj.reshape(1, C).astype(NP_BF16))

    return wqk_arr, wv_aug, wp_t, bvaug, ones, vones, qk_bias, p_bias, extra


def kernel(x, w_qkv, b_qkv, w_proj, b_proj, H=32, W=32):
    x = np.asarray(x, dtype=np.float32)
    assert x.shape == (B, N, C), x.shape
    assert int(H) * int(W) == N

    wqk_arr, wv_aug, wp_t, bvaug, ones, vones, qk_bias, p_bias, extra = _prep_shared(
        w_qkv, b_qkv, w_proj, b_proj
    )
    nc = _build(qk_bias, p_bias)

    in_maps = []
    for c in range(NCORES):
        xc = x[BPC * c : BPC * (c + 1)].reshape(BPC * N, C)
        xT = np.ascontiguousarray(xc.T.astype(NP_BF16))  # [C, BPC*N]
        m = {
            "xT": xT,
            "wqk": wqk_arr,
            "wv": wv_aug,
            "wp": wp_t,
            "bvaug": bvaug,
            "ones": ones,
            "vones": vones,
        }
        m.update(extra)
        in_maps.append(m)

    trace = os.environ.get("KERNEL_TRACE") == "1"
    res = run_bass_kernel_spmd(
        nc, in_maps, core_ids=list(range(NCORES)), trace=trace
    )
    if trace:
        kernel.last_results = res
        print("exec_time_ns:", res.exec_time_ns, "mean:", res.mean_exec_time_ns)
        if res.instructions_and_trace:
            print("trace:", res.instructions_and_trace[1])

    out = np.empty((B, N, C), dtype=np.float32)
    for c in range(NCORES):
        out[BPC * c : BPC * (c + 1)] = res.results[c]["y"]
    return out


if __name__ == "__main__":
    rng = np.random.default_rng(0)
    x = rng.standard_normal((B, N, C), dtype=np.float32)
    w_qkv = rng.standard_normal((3 * C, C), dtype=np.float32) / np.sqrt(C)
    b_qkv = np.zeros(3 * C, np.float32)
    w_proj = rng.standard_normal((C, C), dtype=np.float32) / np.sqrt(C)
    b_proj = np.zeros(C, np.float32)
    y = kernel(x, w_qkv, b_qkv, w_proj, b_proj)
    print("out", y.shape, y.dtype, float(np.abs(y).mean()))
